# revision 25
# baseline (speedup 1.0000x reference)
"""DecisionMetaMamba Trainium2 kernel.

Sharding: data-parallel over batch across 8 NeuronCores (4 batches/core, all
params replicated). Per core:
  - activations in SBUF as [feature(partition), token(free)] fp16, batch as
    the outer loop (4 x 3072 tokens).
  - all GEMMs on TensorE (fp16 operands, fp32 PSUM accumulate); LN gamma/beta
    folded into the consuming GEMM weights host-side.
  - LN stats via ones-matmul on PE; 1/sqrt via ACT Ln+Exp; per-token mu/r
    broadcast across partitions on GpSimd.
  - mamba selective scan: per state index n (16), DVE tensor_tensor_scan
    along time; a_n = exp(A[:,n]*dt) on ScalarE (per-partition scale);
    b_n = (dt*u) * bcast(B_n); y = sum_n h_n * bcast(C_n).

Dispatch path: the compiled program, the jitted PJRT callable, and the
device-resident weight buffers are cached across calls; per call we ship only
the raw fp16 activations, build the interleaved [27, L] token layout on
device with a small XLA jit, and read back the [AD, BC*T] output.

kernel(**inputs) takes full unsharded inputs, returns the full output.
"""

import numpy as np

import jax
import jax.numpy as jnp
from jax.sharding import Mesh, PartitionSpec, NamedSharding
from jax.experimental.shard_map import shard_map

import concourse.bass as bass  # noqa: F401
import concourse.bacc as bacc
import concourse.mybir as mybir
import concourse.tile as tile
from concourse import bass2jax
from concourse.bass_utils import run_bass_kernel_spmd  # noqa: F401

FP32 = mybir.dt.float32
FP16 = mybir.dt.float16
AF = mybir.ActivationFunctionType
ALU = mybir.AluOpType

H = 256
NL = 3
W = 6
DS = 16
DC = 4
DTR = 16
SD = 17
AD = 6
EPS = 1e-5
NCORES = 8

MIXPAD = W - 1   # zero cols before each batch segment for the window mixer
CONVPAD = DC - 1

# experiment-only ablation flags (leave False for correct results)
SKIP_SCAN = False
SKIP_MIX = False
SKIP_BCAST = False
SCAN_AS_MUL = False
SKIP_EXP = False



_TN = [0]


def _tn():
    _TN[0] += 1
    return f"t{_TN[0]}"

def _f16(x):
    return np.asarray(x, np.float32).astype(np.float16)


class ConstMap:
    def __init__(self):
        self.cols = []
        self.idx = {}

    def add(self, name, vec128):
        self.idx[name] = len(self.cols)
        self.cols.append(np.asarray(vec128, np.float32).reshape(128))

    def array(self):
        return np.stack(self.cols, axis=1)


def prep_host(inputs):
    f32 = lambda k: np.asarray(inputs[k], np.float32)
    g1, b1 = f32("g1"), f32("b1")
    g2, b2 = f32("g2"), f32("b2")
    gf, bf = f32("gf"), f32("bf")

    cm = ConstMap()
    for dh in range(2):
        cm.add(f"gemb{dh}", f32("g_emb")[dh * 128:(dh + 1) * 128])
        cm.add(f"bemb{dh}", f32("be_emb")[dh * 128:(dh + 1) * 128])
    weights = {}

    wemb = np.zeros((27, H), np.float32)
    wemb[0, :] = f32("W_ret")[:, 0]
    wemb[1:1 + SD, :] = f32("W_st").T
    wemb[1 + SD:1 + SD + AD, :] = f32("W_act").T
    wemb[24, :] = f32("b_ret")
    wemb[25, :] = f32("b_st")
    wemb[26, :] = f32("b_act")
    weights["wemb"] = _f16(wemb)

    for l in range(NL):
        mixW = f32("mix_W")[l]                       # [H, W*H]
        Wp = mixW.T * g1[np.tile(np.arange(H), W) % H][:, None]
        t = np.zeros((128, 12 * 2 * 128), np.float32)
        for k in range(12):
            for m in range(2):
                t[:, (k * 2 + m) * 128:(k * 2 + m + 1) * 128] = \
                    Wp[k * 128:(k + 1) * 128, m * 128:(m + 1) * 128]
        weights[f"mixw{l}"] = _f16(t)
        bmix = mixW @ np.tile(b1, W)
        for dh in range(2):
            cm.add(f"bmix{l}_{dh}", bmix[dh * 128:(dh + 1) * 128])

        inW = f32("in_W")[l]                         # [2H, H]
        Wp = inW.T * g2[:, None]
        t = np.zeros((128, 2 * 4 * 128), np.float32)
        for k in range(2):
            for m in range(4):
                t[:, (k * 4 + m) * 128:(k * 4 + m + 1) * 128] = \
                    Wp[k * 128:(k + 1) * 128, m * 128:(m + 1) * 128]
        weights[f"inw{l}"] = _f16(t)
        binw = inW @ b2
        for dh in range(2):
            cm.add(f"binxi{l}_{dh}", binw[dh * 128:(dh + 1) * 128])
        for dh in range(2):
            cm.add(f"binz{l}_{dh}", binw[H + dh * 128:H + (dh + 1) * 128])

        cw = f32("conv_w")[l]
        for k in range(DC):
            for dh in range(2):
                cm.add(f"cw{l}_{k}_{dh}", cw[dh * 128:(dh + 1) * 128, k])
        cb = f32("conv_b")[l]
        for dh in range(2):
            cm.add(f"cb{l}_{dh}", cb[dh * 128:(dh + 1) * 128])

        xpW = f32("xproj_W")[l]                      # [48, H]
        t = np.zeros((128, 2 * 48), np.float32)
        for k in range(2):
            t[:, k * 48:(k + 1) * 48] = xpW.T[k * 128:(k + 1) * 128, :]
        weights[f"xpw{l}"] = _f16(t)

        dtW = f32("dt_W")[l]                         # [H, DTR]
        t = np.zeros((DTR, 2 * 128), np.float32)
        for m in range(2):
            t[:, m * 128:(m + 1) * 128] = dtW.T[:, m * 128:(m + 1) * 128]
        weights[f"dtw{l}"] = _f16(t)
        dtb = f32("dt_b")[l]
        for dh in range(2):
            cm.add(f"dtb{l}_{dh}", dtb[dh * 128:(dh + 1) * 128])

        A = -np.exp(f32("A_log")[l])                 # [H, DS]
        for n in range(DS):
            for dh in range(2):
                cm.add(f"A{l}_{n}_{dh}", A[dh * 128:(dh + 1) * 128, n])
        Dp = f32("D_p")[l]
        for dh in range(2):
            cm.add(f"Dp{l}_{dh}", Dp[dh * 128:(dh + 1) * 128])

        outW = f32("out_W")[l]                       # [H, H]
        t = np.zeros((128, 2 * 2 * 128), np.float32)
        for k in range(2):
            for m in range(2):
                t[:, (k * 2 + m) * 128:(k * 2 + m + 1) * 128] = \
                    outW.T[k * 128:(k + 1) * 128, m * 128:(m + 1) * 128]
        weights[f"outw{l}"] = _f16(t)

    paW = f32("pa_W")
    Wp = paW.T * gf[:, None]
    t = np.zeros((128, 2 * AD), np.float32)
    for k in range(2):
        t[:, k * AD:(k + 1) * AD] = Wp[k * 128:(k + 1) * 128, :]
    weights["paw"] = _f16(t)
    pab = f32("pa_b") + paW @ bf
    pabcol = np.zeros(128, np.float32)
    pabcol[:AD] = pab
    cm.add("pab", pabcol)

    return weights, cm


# ---------------------------------------------------------------------------
def build_program(BC, T, cm_idx, n_const_cols):
    L = 3 * T
    CH = min(512, L)
    NCH = L // CH
    NTH = 2 if L >= 2048 else 1
    TH = L // NTH
    CHT = TH // CH

    nc = bacc.Bacc("TRN2", target_bir_lowering=False, debug=False,
                   enable_asserts=False)

    x_d = nc.dram_tensor("x", [27, BC * L], FP16, kind="ExternalInput").ap()
    consts_d = nc.dram_tensor("consts", [128, n_const_cols], FP32,
                              kind="ExternalInput").ap()
    wd = {}
    for name, shape in (
        [("wemb", [27, 256]), ("paw", [128, 2 * AD])]
        + [(f"mixw{l}", [128, 12 * 2 * 128]) for l in range(NL)]
        + [(f"inw{l}", [128, 2 * 4 * 128]) for l in range(NL)]
        + [(f"xpw{l}", [128, 2 * 48]) for l in range(NL)]
        + [(f"dtw{l}", [DTR, 2 * 128]) for l in range(NL)]
        + [(f"outw{l}", [128, 2 * 2 * 128]) for l in range(NL)]
    ):
        wd[name] = nc.dram_tensor(name, shape, FP16, kind="ExternalInput").ap()
    out_d = nc.dram_tensor("out", [AD, BC * T], FP16,
                           kind="ExternalOutput").ap()

    with tile.TileContext(nc) as tc:
        _kern(tc, nc, x_d, consts_d, wd, out_d, BC, T, L, CH, NCH, NTH, TH,
              CHT, cm_idx)
    nc.compile()
    return nc


def _kern(tc, nc, x_d, consts_d, wd, out_d, BC, T, L, CH, NCH, NTH, TH, CHT,
          cm_idx):
    _pool_cms = []

    def pool(name, bufs, space="SBUF"):
        cm = tc.tile_pool(name=name, bufs=bufs, space=space)
        _pool_cms.append(cm)
        return cm.__enter__()

    wpool = pool("w", 1)
    consts = wpool.tile([128, n_cols(cm_idx)], FP32, name="consts",
                        tag="consts")
    nc.sync.dma_start(consts[:, :], consts_d[:, :])

    def ccol(name):
        j = cm_idx[name]
        return consts[:, j:j + 1]

    # small weights stay resident; big per-layer weights stream via a pool
    wsb = {}
    for name in ["wemb", "paw"] + [f"xpw{l}" for l in range(NL)] + \
            [f"dtw{l}" for l in range(NL)]:
        ap = wd[name]
        t = wpool.tile(list(ap.shape), FP16, name=name, tag=name)
        nc.sync.dma_start(t[:, :], ap[:, :])
        wsb[name] = t

    ones_lhs = wpool.tile([128, 1], FP16, name="ones", tag="ones")
    nc.vector.memset(ones_lhs[:, :], 1.0 / H)

    wbig_pool = pool("wbig", 1)
    hs_pool = pool("hs", 2)
    xpad_pool = pool("xpad", 2)
    big_pool = pool("big", 1)            # sz (full-L)
    tht_pool = pool("tht", 1)            # xc/dt/dtu per-half
    scan_pool = pool("scan", 2)
    acc_pool = pool("acc", 1)
    bc_pool = pool("bc", 2)
    lnb_pool = pool("lnb", 2)
    stat_pool = pool("stat", 2)
    small_pool = pool("small", 2)
    xin_pool = pool("xin", 2)
    out_pool = pool("outp", 1)
    ps_pool = pool("ps", 4, space="PSUM")
    ps_stat = pool("pst", 2, space="PSUM")

    def matmul_acc(psum, lhs_list, rhs_list):
        for i, (lh, rh) in enumerate(zip(lhs_list, rhs_list)):
            nc.tensor.matmul(psum, lh, rh, start=(i == 0),
                             stop=(i == len(lhs_list) - 1))

    def layernorm(src, dst, affine=None):
        """src/dst: per-dh lists of [128, L] APs. LN over the feature dim.

        Works chunk-by-chunk: stats via ones-matmul on PE, r = exp(-ln/2)
        on ACT, mu/r broadcast on GpSimd, apply via DVE (in-place mul)."""
        for ch in range(NCH):
            sl = slice(ch * CH, (ch + 1) * CH)
            ps = ps_stat.tile([33, CH], FP32, name=_tn(), tag="lnstat")
            sq0 = stat_pool.tile([128, CH], FP16, name=_tn(), tag="sq")
            sq1 = stat_pool.tile([128, CH], FP16, name=_tn(), tag="sq")
            nc.vector.tensor_mul(sq0[:, :], src[0][:, sl], src[0][:, sl])
            nc.vector.tensor_mul(sq1[:, :], src[1][:, sl], src[1][:, sl])
            matmul_acc(ps[0:1, :], [ones_lhs[:, :]] * 2,
                       [src[0][:, sl], src[1][:, sl]])
            matmul_acc(ps[32:33, :], [ones_lhs[:, :]] * 2,
                       [sq0[:, :], sq1[:, :]])
            mu16 = stat_pool.tile([1, CH], FP16, name=_tn(), tag="mu16", bufs=1)
            nc.vector.tensor_scalar_mul(mu16[:, :], ps[0:1, :], 1.0)
            mu2 = stat_pool.tile([1, CH], FP32, name=_tn(), tag="mu2", bufs=1)
            nc.scalar.activation(mu2[:, :], ps[0:1, :], AF.Square)
            var = stat_pool.tile([1, CH], FP32, name=_tn(), tag="var", bufs=1)
            nc.vector.scalar_tensor_tensor(var[:, :], ps[32:33, :], EPS,
                                           mu2[:, :], ALU.add, ALU.subtract)
            nc.scalar.activation(var[:, :], var[:, :], AF.Ln)
            r16 = stat_pool.tile([1, CH], FP16, name=_tn(), tag="r16", bufs=1)
            nc.scalar.activation(r16[:, :], var[:, :], AF.Exp, scale=-0.5)
            mu_b = lnb_pool.tile([128, CH], FP16, name=_tn(), tag="mub")
            r_b = lnb_pool.tile([128, CH], FP16, name=_tn(), tag="rb")
            nc.gpsimd.partition_broadcast(mu_b[:, :], mu16[:, :])
            nc.gpsimd.partition_broadcast(r_b[:, :], r16[:, :])
            for dh in range(2):
                d = dst[dh][:, sl] if dst[dh].shape[1] == L else dst[dh]
                nc.vector.tensor_sub(d, src[dh][:, sl], mu_b[:, :])
                nc.vector.tensor_mul(d, d, r_b[:, :])
                if affine is not None:
                    gname, bname = affine
                    nc.scalar.activation(d, d, AF.Identity,
                                         bias=ccol(f"{bname}{dh}"),
                                         scale=ccol(f"{gname}{dh}"))

    # ================= per batch =================
    for b in range(BC):
        boff = b * L

        he = [xpad_pool.tile([128, L + MIXPAD], FP16, name=_tn(),
                             tag=f"pad{dh}") for dh in range(2)]
        for ch in range(NCH):
            xin = xin_pool.tile([27, CH], FP16, name=_tn(), tag="xin")
            nc.sync.dma_start(xin[:, :],
                              x_d[:, boff + ch * CH:boff + (ch + 1) * CH])
            for m in range(2):
                ps = ps_pool.tile([128, CH], FP32, name=_tn(), tag="ps")
                nc.tensor.matmul(ps[:, :],
                                 wsb["wemb"][:, m * 128:(m + 1) * 128],
                                 xin[:, :], start=True, stop=True)
                nc.scalar.activation(
                    he[m][:, MIXPAD + ch * CH:MIXPAD + (ch + 1) * CH],
                    ps[:, :], AF.Copy)
        hs = [hs_pool.tile([128, L], FP16, name=_tn(), tag=f"hs{dh}")
              for dh in range(2)]
        layernorm([he[dh][:, MIXPAD:] for dh in range(2)],
                  [hs[dh][:, :] for dh in range(2)],
                  affine=("gemb", "bemb"))

        for l in range(NL):
            # ---- LN1 -> x1 (padded) ----
            x1 = [xpad_pool.tile([128, L + MIXPAD], FP16, name=_tn(),
                                 tag=f"pad{dh}") for dh in range(2)]
            for dh in range(2):
                nc.vector.memset(x1[dh][:, 0:MIXPAD], 0.0)
            layernorm([hs[dh][:, :] for dh in range(2)],
                      [x1[dh][:, MIXPAD:] for dh in range(2)])

            # ---- dense mix + residual ----
            mw = wbig_pool.tile([128, 12 * 2 * 128], FP16, name=_tn(),
                                tag="mixw")
            nc.sync.dma_start(mw[:, :], wd[f"mixw{l}"][:, :])
            for ch in range(NCH if not SKIP_MIX else 0):
                sl = slice(ch * CH, (ch + 1) * CH)
                for m in range(2):
                    ps = ps_pool.tile([128, CH], FP32, name=_tn(), tag="ps")
                    lhs, rhs = [], []
                    for w in range(W):
                        for k in range(2):
                            kk = w * 2 + k
                            lhs.append(mw[:, (kk * 2 + m) * 128:
                                          (kk * 2 + m + 1) * 128])
                            rhs.append(x1[k][:, w + ch * CH:w + (ch + 1) * CH])
                    matmul_acc(ps[:, :], lhs, rhs)
                    nc.vector.scalar_tensor_tensor(
                        hs[m][:, sl], ps[:, :], ccol(f"bmix{l}_{m}"),
                        hs[m][:, sl], ALU.add, ALU.add)

            # ---- LN2 -> x2 ----
            x2 = [xpad_pool.tile([128, L + MIXPAD], FP16, name=_tn(),
                                 tag=f"pad{dh}") for dh in range(2)]
            layernorm([hs[dh][:, :] for dh in range(2)],
                      [x2[dh][:, 0:L] for dh in range(2)])

            # ---- in_proj -> xi (padded+bias), sz (silu) ----
            iw = wbig_pool.tile([128, 2 * 4 * 128], FP16, name=_tn(),
                                tag="inw")
            nc.sync.dma_start(iw[:, :], wd[f"inw{l}"][:, :])
            xi = [xpad_pool.tile([128, L + MIXPAD], FP16, name=_tn(),
                                 tag=f"pad{dh}") for dh in range(2)]
            for dh in range(2):
                nc.vector.memset(xi[dh][:, 0:CONVPAD], 0.0)
            sz = [big_pool.tile([128, L], FP16, name=_tn(), tag=f"sz{dh}")
                  for dh in range(2)]
            for ch in range(NCH):
                sl = slice(ch * CH, (ch + 1) * CH)
                for m in range(4):
                    ps = ps_pool.tile([128, CH], FP32, name=_tn(), tag="ps")
                    lhs = [iw[:, (k * 4 + m) * 128:(k * 4 + m + 1) * 128]
                           for k in range(2)]
                    rhs = [x2[k][:, sl] for k in range(2)]
                    matmul_acc(ps[:, :], lhs, rhs)
                    if m < 2:
                        nc.scalar.activation(
                            xi[m][:, CONVPAD + ch * CH:CONVPAD + (ch + 1) * CH],
                            ps[:, :], AF.Identity, bias=ccol(f"binxi{l}_{m}"))
                    else:
                        zt = stat_pool.tile([128, CH], FP16, name=_tn(),
                                            tag="zt")
                        sg = stat_pool.tile([128, CH], FP16, name=_tn(),
                                            tag="sg")
                        nc.scalar.activation(zt[:, :], ps[:, :], AF.Identity,
                                             bias=ccol(f"binz{l}_{m - 2}"))
                        nc.scalar.activation(sg[:, :], ps[:, :], AF.Sigmoid,
                                             bias=ccol(f"binz{l}_{m - 2}"))
                        nc.vector.tensor_mul(sz[m - 2][:, sl], zt[:, :],
                                             sg[:, :])

            ow = wbig_pool.tile([128, 2 * 2 * 128], FP16, name=_tn(),
                                tag="outw")
            nc.sync.dma_start(ow[:, :], wd[f"outw{l}"][:, :])
            xw = wsb[f"xpw{l}"]
            dw = wsb[f"dtw{l}"]
            carry = [small_pool.tile([128, DS], FP32, name=_tn(),
                                     tag=f"carry{dh}") for dh in range(2)]

            for th in range(NTH):
                tsl = slice(th * TH, (th + 1) * TH)
                # conv + silu -> xc (per half; xi is full-L so shifted reads
                # cross the half boundary correctly)
                xc = [tht_pool.tile([128, TH], FP16, name=_tn(),
                                    tag=f"xc{dh}") for dh in range(2)]
                for dh in range(2):
                    acc = stat_pool.tile([128, TH], FP16, name=_tn(),
                                         tag="cacc", bufs=1)
                    nc.vector.tensor_scalar_mul(
                        acc[:, :], xi[dh][:, th * TH:th * TH + TH],
                        ccol(f"cw{l}_0_{dh}"))
                    for k in range(1, DC):
                        nc.vector.scalar_tensor_tensor(
                            acc[:, :], xi[dh][:, th * TH + k:th * TH + k + TH],
                            ccol(f"cw{l}_{k}_{dh}"), acc[:, :],
                            ALU.mult, ALU.add)
                    ct = stat_pool.tile([128, TH], FP16, name=_tn(), tag="ct", bufs=1)
                    cg = stat_pool.tile([128, TH], FP16, name=_tn(), tag="cg", bufs=1)
                    nc.scalar.activation(ct[:, :], acc[:, :], AF.Identity,
                                         bias=ccol(f"cb{l}_{dh}"))
                    nc.scalar.activation(cg[:, :], acc[:, :], AF.Sigmoid,
                                         bias=ccol(f"cb{l}_{dh}"))
                    nc.vector.tensor_mul(xc[dh][:, :], ct[:, :], cg[:, :])

                # xproj -> xdb [48, TH]
                xdb = small_pool.tile([48, TH], FP16, name=_tn(), tag="xdb")
                for ch in range(CHT):
                    slh = slice(ch * CH, (ch + 1) * CH)
                    ps = ps_pool.tile([48, CH], FP32, name=_tn(), tag="ps")
                    matmul_acc(ps[:, :], [xw[:, 0:48], xw[:, 48:96]],
                               [xc[0][:, slh], xc[1][:, slh]])
                    nc.scalar.activation(xdb[:, slh], ps[:, :], AF.Copy)

                # dt = softplus = ln(exp(raw + dtb) + 1); dtu = dt*xc
                dt = [tht_pool.tile([128, TH], FP16, name=_tn(),
                                    tag=f"dt{dh}") for dh in range(2)]
                dtu = [tht_pool.tile([128, TH], FP16, name=_tn(),
                                     tag=f"dtu{dh}") for dh in range(2)]
                for ch in range(CHT):
                    slh = slice(ch * CH, (ch + 1) * CH)
                    for m in range(2):
                        ps = ps_pool.tile([128, CH], FP32, name=_tn(),
                                          tag="ps")
                        nc.tensor.matmul(ps[:, :],
                                         dw[:, m * 128:(m + 1) * 128],
                                         xdb[0:DTR, slh], start=True,
                                         stop=True)
                        nc.scalar.activation(ps[:, :], ps[:, :], AF.Exp,
                                             bias=ccol(f"dtb{l}_{m}"))
                        nc.scalar.activation(dt[m][:, slh], ps[:, :], AF.Ln,
                                             bias=1.0)
                for dh in range(2):
                    nc.vector.tensor_mul(dtu[dh][:, :], dt[dh][:, :],
                                         xc[dh][:, :])

                # ---- 16 state scans ----
                yacc = [acc_pool.tile([128, TH], FP16, name=_tn(),
                                      tag=f"ya{dh}") for dh in range(2)]
                if SKIP_SCAN:
                    for dh in range(2):
                        nc.vector.memset(yacc[dh][:, :], 0.0)
                for n in range(DS if not SKIP_SCAN else 0):
                    if SKIP_BCAST:
                        B_b, C_b = dt[0][:, :], dt[1][:, :]
                    else:
                        brow = small_pool.tile([1, 2 * TH], FP16, name=_tn(),
                                               tag="brow")
                        nc.sync.dma_start(
                            brow[:, :], xdb[DTR + n:DTR + DS + n + 1:DS, :])
                        BC_b = bc_pool.tile([128, 2 * TH], FP16, name=_tn(),
                                            tag="BCb")
                        nc.gpsimd.partition_broadcast(BC_b[:, :], brow[:, :])
                        B_b = BC_b[:, 0:TH]
                        C_b = BC_b[:, TH:2 * TH]
                    for dh in range(2):
                        a = scan_pool.tile([128, TH], FP16, name=_tn(),
                                           tag="a")
                        if SKIP_EXP:
                            a = dt[dh]
                        else:
                            nc.scalar.activation(a[:, :], dt[dh][:, :],
                                                 AF.Exp,
                                                 scale=ccol(f"A{l}_{n}_{dh}"))
                        bb = scan_pool.tile([128, TH], FP16, name=_tn(),
                                            tag="bb")
                        nc.vector.tensor_mul(bb[:, :], dtu[dh][:, :], B_b)
                        h = scan_pool.tile([128, TH], FP16, name=_tn(),
                                           tag="h")
                        init = 0.0 if th == 0 else carry[dh][:, n:n + 1]
                        if SCAN_AS_MUL:
                            nc.vector.tensor_mul(h[:, :], a[:, :], bb[:, :])
                        else:
                            nc.vector.tensor_tensor_scan(h[:, :], a[:, :],
                                                         bb[:, :], init,
                                                         ALU.mult, ALU.add)
                        if NTH == 2 and th == 0:
                            nc.vector.tensor_copy(carry[dh][:, n:n + 1],
                                                  h[:, TH - 1:TH])
                        if n == 0:
                            nc.vector.tensor_mul(yacc[dh][:, :], h[:, :], C_b)
                        else:
                            q = scan_pool.tile([128, TH], FP16, name=_tn(),
                                               tag="q")
                            nc.vector.tensor_mul(q[:, :], h[:, :], C_b)
                            nc.vector.tensor_add(yacc[dh][:, :],
                                                 yacc[dh][:, :], q[:, :])

                # ---- gate + out_proj + residual ----
                g = [None, None]
                for dh in range(2):
                    yt = scan_pool.tile([128, TH], FP16, name=_tn(),
                                        tag="yt", bufs=1)
                    nc.vector.scalar_tensor_tensor(
                        yt[:, :], xc[dh][:, :], ccol(f"Dp{l}_{dh}"),
                        yacc[dh][:, :], ALU.mult, ALU.add)
                    gg = scan_pool.tile([128, TH], FP16, name=_tn(),
                                        tag=f"g{dh}", bufs=1)
                    nc.vector.tensor_mul(gg[:, :], yt[:, :], sz[dh][:, tsl])
                    g[dh] = gg
                for ch in range(CHT):
                    sl_h = slice(ch * CH, (ch + 1) * CH)
                    sl_g = slice(th * TH + ch * CH, th * TH + (ch + 1) * CH)
                    for m in range(2):
                        ps = ps_pool.tile([128, CH], FP32, name=_tn(),
                                          tag="ps")
                        lhs = [ow[:, (k * 2 + m) * 128:(k * 2 + m + 1) * 128]
                               for k in range(2)]
                        rhs = [g[k][:, sl_h] for k in range(2)]
                        matmul_acc(ps[:, :], lhs, rhs)
                        nc.vector.scalar_tensor_tensor(
                            hs[m][:, sl_g], ps[:, :], 0.0, hs[m][:, sl_g],
                            ALU.add, ALU.add)

        # ---- final LN + head ----
        xf = [xpad_pool.tile([128, L + MIXPAD], FP16, name=_tn(),
                             tag=f"pad{dh}") for dh in range(2)]
        layernorm([hs[dh][:, :] for dh in range(2)],
                  [xf[dh][:, 0:L] for dh in range(2)])
        pw = wsb["paw"]
        outsb = out_pool.tile([AD, T], FP16, name=_tn(), tag="out")
        TCH = min(512, T)
        for ch in range(T // TCH):
            ps = ps_pool.tile([AD, TCH], FP32, name=_tn(), tag="ps")
            lhs = [pw[:, k * AD:(k + 1) * AD] for k in range(2)]
            rhs = []
            for k in range(2):
                xfr = xf[k][:, 0:L].rearrange("p (t c) -> p t c", c=3)
                rhs.append(xfr[:, ch * TCH:(ch + 1) * TCH, 1])
            matmul_acc(ps[:, :], lhs, rhs)
            nc.scalar.activation(outsb[:, ch * TCH:(ch + 1) * TCH], ps[:, :],
                                 AF.Tanh, bias=ccol("pab")[0:AD, :])
        nc.sync.dma_start(out_d[:, b * T:(b + 1) * T], outsb[:, :])


def n_cols(cm_idx):
    return max(cm_idx.values()) + 1


# ---------------------------------------------------------------------------
# Dispatch: cached jitted PJRT callable + device-resident weights.

class _Ctx:
    pass


_CTX_CACHE = {}
TRACE = False
LAST_RESULTS = None


class _FakeResults:
    """test.py compatibility: exec_time_ns unavailable under axon."""
    exec_time_ns = None
    mean_exec_time_ns = None
    instructions_and_trace = None
    profile_json = None

    def __init__(self, results):
        self.results = results


def _get_ctx(BC, T, cm_idx, n_const_cols):
    key = (BC, T)
    if key in _CTX_CACHE:
        return _CTX_CACHE[key]

    ctx = _Ctx()
    ctx.nc = build_program(BC, T, cm_idx, n_const_cols)
    nc = ctx.nc
    bass2jax.install_neuronx_cc_hook()

    partition_name = (nc.partition_id_tensor.name
                      if nc.partition_id_tensor else None)
    in_names, out_names, out_avals = [], [], []
    for alloc in nc.m.functions[0].allocations:
        if not isinstance(alloc, mybir.MemoryLocationSet):
            continue
        name = alloc.memorylocations[0].name
        if alloc.kind == "ExternalInput":
            if name != partition_name:
                in_names.append(name)
        elif alloc.kind == "ExternalOutput":
            shape = tuple(alloc.tensor_shape)
            dtype = mybir.dt.np(alloc.dtype)
            out_names.append(name)
            out_avals.append(jax.core.ShapedArray(shape, dtype))
    n_params = len(in_names)
    in_names_all = in_names + out_names + (
        [partition_name] if partition_name else [])

    def _body(*args):
        operands = list(args)
        if partition_name is not None:
            operands.append(bass2jax.partition_id_tensor())
        outs = bass2jax._bass_exec_p.bind(
            *operands, out_avals=tuple(out_avals),
            in_names=tuple(in_names_all), out_names=tuple(out_names),
            lowering_input_output_aliases=(),
            sim_require_finite=True, sim_require_nnan=True, nc=nc)
        return tuple(outs)

    devices = jax.devices()[:NCORES]
    mesh = Mesh(np.asarray(devices), ("core",))
    n_outs = len(out_names)
    ctx.mesh = mesh
    ctx.sharding = NamedSharding(mesh, PartitionSpec("core"))
    ctx.in_names = in_names
    ctx.out_names = out_names
    ctx.out_avals = out_avals
    ctx.bass_fn = jax.jit(
        shard_map(_body, mesh=mesh,
                  in_specs=(PartitionSpec("core"),) * (n_params + n_outs),
                  out_specs=(PartitionSpec("core"),) * n_outs,
                  check_rep=False),
        keep_unused=True)

    L = 3 * T

    def _pack_local(xc):
        # xc [BC,T,24] fp16 (local shard): rtg | states | actions
        rt, st, ac = xc[..., 0:1], xc[..., 1:1 + SD], xc[..., 1 + SD:]
        z = lambda k: jnp.zeros((BC, T, k), jnp.float16)
        one = jnp.ones((BC, T, 1), jnp.float16)
        p0 = jnp.concatenate([rt, z(23), one, z(2)], axis=-1)       # rtg row
        p1 = jnp.concatenate([z(1), st, z(6 + 1), one, z(1)], axis=-1)
        p2 = jnp.concatenate([z(18), ac, z(2), one], axis=-1)
        x = jnp.stack([p0, p1, p2], axis=2)          # [BC,T,3,27]
        return x.transpose(3, 0, 1, 2).reshape(27, BC * L)

    ctx.pack_fn = jax.jit(
        shard_map(_pack_local, mesh=mesh,
                  in_specs=(PartitionSpec("core"),),
                  out_specs=PartitionSpec("core")))

    # device-resident zero output operands (kernel writes every output
    # element, so these are never donated and can be reused across calls)
    ctx.dev_zeros = [
        jax.device_put(
            np.zeros((NCORES * a.shape[0], *a.shape[1:]), a.dtype),
            ctx.sharding)
        for a in out_avals]
    ctx.dev_weights = None     # name -> device array
    ctx.host_raw = None        # raw weight inputs (for change detection)

    _CTX_CACHE[key] = ctx
    return ctx


# every input that feeds the packed weights / consts (not activations)
_W_KEYS = ("W_ret", "b_ret", "W_st", "b_st", "W_act", "b_act", "g_emb",
           "be_emb", "g1", "b1", "g2", "b2", "gf", "bf", "mix_W", "in_W",
           "conv_w", "conv_b", "xproj_W", "dt_W", "dt_b", "A_log", "D_p",
           "out_W", "pa_W", "pa_b")


def _ensure_weights(ctx, inputs):
    raw = {k: np.asarray(inputs[k], np.float32) for k in _W_KEYS}
    if ctx.host_raw is not None and all(
            np.array_equal(ctx.host_raw[k], raw[k]) for k in _W_KEYS):
        return
    weights, cm = prep_host(inputs)
    base = dict(weights)
    base["consts"] = np.ascontiguousarray(cm.array())
    ctx.dev_weights = {
        name: jax.device_put(np.concatenate([arr] * NCORES, axis=0),
                             ctx.sharding)
        for name, arr in base.items()}
    jax.block_until_ready(list(ctx.dev_weights.values()))
    ctx.host_raw = {k: v.copy() for k, v in raw.items()}


def kernel(**inputs):
    states = np.asarray(inputs["states"], np.float32)
    actions = np.asarray(inputs["actions"], np.float32)
    rtg = np.asarray(inputs["returns_to_go"], np.float32)
    B, T = states.shape[0], states.shape[1]
    BC = B // NCORES

    key = (BC, T)
    ctx = _CTX_CACHE.get(key)
    if ctx is None:
        weights, cm = prep_host(inputs)
        ctx = _get_ctx(BC, T, cm.idx, cm.array().shape[1])

    # ship activations first so the H2D stream overlaps any host-side work
    xcat = np.empty((B, T, 1 + SD + AD), np.float16)  # [B,T,24]
    xcat[..., 0:1] = rtg
    xcat[..., 1:1 + SD] = states
    xcat[..., 1 + SD:] = actions
    xdev = ctx.pack_fn(xcat)

    _ensure_weights(ctx, inputs)

    args = [xdev if nm == "x" else ctx.dev_weights[nm]
            for nm in ctx.in_names]
    outs = ctx.bass_fn(*args, *ctx.dev_zeros)

    global LAST_RESULTS
    out_np = np.asarray(outs[0])                      # [8*AD, BC*T] fp16
    LAST_RESULTS = _FakeResults(
        [{"out": out_np.reshape(NCORES, AD, BC * T)[c]}
         for c in range(NCORES)])
    out = out_np.reshape(NCORES, AD, BC, T).transpose(0, 2, 3, 1)
    return np.ascontiguousarray(out.reshape(B, T, AD)).astype(np.float32)
